# revision 7
# baseline (speedup 1.0000x reference)
"""Trainium2 Bass kernel for nn_LocalNetwork (avgpool3d -> 3x LocallyConnected1D -> upsample3d).

Sharding: pure data parallelism — batch 256 split as 32 per core across 8 cores.

Per-core layout (B_loc=32 batches, 4 load groups of 8, conv pairs of 2 groups):
  partition p = (bl, dslice)  [8 x 15 = 120 partitions]
  Every DMA descriptor covers a 32KB-contiguous DRAM run (one (h,w) slice).

  - avg-pool over (ws) then (hs): 3+3 tensor_tensor adds (DVE runs TT at
    ~2.4 elem/cycle vs 1 for tensor_reduce)
  - depth pool + depth-conv taps: matmuls [120 -> 40] with 0/(1/48)
    matrices (fuses the /48 mean scale and the +/-1 depth shifts)
  - two load-groups pair into [80, 512] conv tiles so the ~680ns/instr
    DVE overhead is amortized over 16 batches
  - upsample: relu + h-expand into ws=0 lanes of U (scalar engine),
    w-expand ws=1..3 in place (vector, broadcast src); depth x3
    replication is FREE — three store DMAs per group read the same
    [40, 8192] partition-slice of U into interleaved DRAM depth slices.
  - loads issue on the sync HWDGE queue, stores on the scalar (Act)
    HWDGE queue: a store waiting for compute must not head-of-line
    block later loads.
"""

import numpy as np

import concourse.bass as bass
import concourse.mybir as mybir
from concourse import bacc
from concourse.bass_utils import run_bass_kernel_spmd
from concourse.tile import TileContext

F32 = mybir.dt.float32
ADD = mybir.AluOpType.add
MULT = mybir.AluOpType.mult
RELU = mybir.ActivationFunctionType.Relu

N_CORES = 8
B = 256
B_CORE = 32          # batches per core
G = 4                # load groups per core
B_GRP = 8            # batches per group
CORE_ELEMS = B_CORE * 15 * 64 * 128  # 3,932,160
BSTRIDE = 15 * 64 * 128              # 122,880
SLICE = 64 * 128                     # 8192 elems = one (h,w) plane = 32KB


def _pack_consts(w_depth, b_depth, w_lon, b_lon, w_lat, b_lat):
    """Returns (mm [120,128] f32, wts [104,6144] f32).

    mm: three matmul lhsT tiles [120,40] (cols 0:40 dn / 40:80 mid / 80:120 up)
        out[q=(bl,dp), f] = sum_p lhsT[p=(bl,dsl), q] * P2[p, f]
        coefficient 1/48 folds the avg-pool mean.
    wts: 12 x [80,512] conv weight/bias tiles, p=(half,bl,dp), f=(ho,wo);
        identical content on partitions 0:40 and 64:104 (conv pairs;
        matmul out base partition must be 0/32/64, so half b sits at 64).
    """
    mm = np.zeros((120, 128), np.float32)
    for bl in range(8):
        for dsl in range(15):
            p = bl * 15 + dsl
            grp = dsl // 3
            for col0, dp in ((0, grp + 1), (40, grp), (80, grp - 1)):
                if 0 <= dp <= 4:
                    mm[p, col0 + bl * 5 + dp] = 1.0 / 48.0

    dp = np.arange(5)[:, None, None]
    ho = np.arange(16)[None, :, None]
    wo = np.arange(32)[None, None, :]
    ld = wo * 112 + ho * 7 + (dp + 1)     # depth seq index (5,16,32)
    ll = dp * 544 + ho * 34 + (wo + 1)    # lon
    lt = dp * 576 + wo * 18 + (ho + 1)    # lat

    def tile(vec, idx):
        t = np.broadcast_to(np.asarray(vec)[idx][None], (8, 5, 16, 32))
        return t.reshape(40, 512)

    cols = []
    for j in range(3):
        cols.append(tile(np.asarray(w_depth)[:, j], ld))
    cols.append(tile(b_depth, ld))
    for j in range(3):
        cols.append(tile(np.asarray(w_lon)[:, j], ll))
    cols.append(tile(b_lon, ll))
    for j in range(3):
        cols.append(tile(np.asarray(w_lat)[:, j], lt))
    cols.append(tile(b_lat, lt))
    wts40 = np.concatenate(cols, axis=1)
    wts = np.zeros((104, 6144), np.float32)
    wts[0:40] = wts40
    wts[64:104] = wts40
    return mm, np.ascontiguousarray(wts)


def build_nc(reps: int = 1) -> bass.Bass:
    nc = bacc.Bacc("TRN2", target_bir_lowering=False, debug=False)
    x = nc.dram_tensor("x", [CORE_ELEMS], F32, kind="ExternalInput")
    mmc = nc.dram_tensor("mm", [120, 128], F32, kind="ExternalInput")
    wtc = nc.dram_tensor("wts", [104, 6144], F32, kind="ExternalInput")
    y = nc.dram_tensor("y", [CORE_ELEMS], F32, kind="ExternalOutput")

    with TileContext(nc) as tc:
        with (
            tc.tile_pool(name="cpool", bufs=1) as cpool,
            tc.tile_pool(name="inp", bufs=2) as inp,
            tc.tile_pool(name="outp", bufs=2) as outp,
            tc.tile_pool(name="work", bufs=2) as work,
            tc.tile_pool(name="psum", bufs=2, space="PSUM") as psum,
        ):
            MM = cpool.tile([120, 128], F32)
            WT = cpool.tile([104, 6144], F32)
            nc.sync.dma_start(MM[:], mmc[:])
            nc.sync.dma_start(WT[:], wtc[:])
            w = lambda i: WT[:, i * 512:(i + 1) * 512]
            wd0, wd1, wd2, bd = (w(i) for i in range(4))
            vl0, vl1, vl2, blon = (w(i) for i in range(4, 8))
            ul0, ul1, ul2, blat = (w(i) for i in range(8, 12))

            state = {}

            def load(g):
                off = (g % G) * B_GRP * BSTRIDE
                X = inp.tile([120, SLICE], F32)
                nc.sync.dma_start(
                    X[:], bass.AP(x, off, [[BSTRIDE, 8], [SLICE, 15], [1, SLICE]]))
                state[g] = X

            def pool(g):
                X = state.pop(g)
                # w-pool: sum ws groups of 4 -> P1 (h, wo)
                Xh = X[:].rearrange("p (h wo ws) -> p h wo ws", h=64, wo=32, ws=4)
                P1 = work.tile([120, 2048], F32)
                P1h = P1[:].rearrange("p (h wo) -> p h wo", h=64)
                nc.vector.tensor_tensor(P1h, Xh[:, :, :, 0], Xh[:, :, :, 1], ADD)
                nc.vector.tensor_tensor(P1h, P1h, Xh[:, :, :, 2], ADD)
                nc.vector.tensor_tensor(P1h, P1h, Xh[:, :, :, 3], ADD)
                # h-pool: sum hs groups of 4 -> P2 (ho, wo)
                P1q = P1[:].rearrange("p (ho hs wo) -> p ho hs wo", ho=16, hs=4)
                P2 = work.tile([120, 512], F32)
                P2h = P2[:].rearrange("p (ho wo) -> p ho wo", ho=16)
                nc.vector.tensor_tensor(P2h, P1q[:, :, 0], P1q[:, :, 1], ADD)
                nc.vector.tensor_tensor(P2h, P2h, P1q[:, :, 2], ADD)
                nc.vector.tensor_tensor(P2h, P2h, P1q[:, :, 3], ADD)
                state[("P2", g)] = P2

            def mm_pair(k):
                # depth pool (/48) + conv taps; pair halves at partitions 0:40 / 40:80
                Sdn = psum.tile([104, 512], F32)
                S0 = psum.tile([104, 512], F32)
                Sup = psum.tile([104, 512], F32)
                for half, g in enumerate((2 * k, 2 * k + 1)):
                    P2 = state.pop(("P2", g))
                    sl = slice(64 * half, 64 * half + 40)
                    nc.tensor.matmul(Sdn[sl], MM[:, 0:40], P2[:], start=True, stop=True)
                    nc.tensor.matmul(S0[sl], MM[:, 40:80], P2[:], start=True, stop=True)
                    nc.tensor.matmul(Sup[sl], MM[:, 80:120], P2[:], start=True, stop=True)
                state[("S", k)] = (Sdn, S0, Sup)

            def conv_store_pair(k):
                Sdn, S0, Sup = state.pop(("S", k))
                # depth conv
                m = work.tile([104, 512], F32)
                m2 = work.tile([104, 512], F32)
                nc.vector.tensor_tensor(m[:], wd0, Sdn[:], MULT)
                nc.vector.tensor_tensor(m2[:], wd1, S0[:], MULT)
                nc.vector.tensor_tensor(m[:], m[:], m2[:], ADD)
                nc.vector.tensor_tensor(m2[:], wd2, Sup[:], MULT)
                nc.vector.tensor_tensor(m[:], m[:], m2[:], ADD)
                nc.vector.tensor_tensor(m[:], m[:], bd, ADD)
                # relu into lon-padded tile Ydp[p, ho*34 + (wo+1)]
                Ydp = work.tile([104, 544], F32)
                Ydpv = Ydp[:].rearrange("p (ho wp) -> p ho wp", ho=16, wp=34)
                nc.gpsimd.memset(Ydpv[:, :, 0], 0)
                nc.gpsimd.memset(Ydpv[:, :, 33], 0)
                nc.vector.tensor_scalar_max(
                    Ydpv[:, :, 1:33],
                    m[:].rearrange("p (ho wo) -> p ho wo", ho=16), 0.0)

                # lon conv (along wo, free axis)
                m3 = m[:].rearrange("p (ho wo) -> p ho wo", ho=16)
                m23 = m2[:].rearrange("p (ho wo) -> p ho wo", ho=16)
                w3 = lambda t: t.rearrange("p (ho wo) -> p ho wo", ho=16)
                nc.vector.tensor_tensor(m3, w3(vl0), Ydpv[:, :, 0:32], MULT)
                nc.vector.tensor_tensor(m23, w3(vl1), Ydpv[:, :, 1:33], MULT)
                nc.vector.tensor_tensor(m3, m3, m23, ADD)
                nc.vector.tensor_tensor(m23, w3(vl2), Ydpv[:, :, 2:34], MULT)
                nc.vector.tensor_tensor(m3, m3, m23, ADD)
                nc.vector.tensor_tensor(m3, m3, w3(blon), ADD)
                # relu into lat-padded tile Ylp[p, (ho+1)*32 + wo]
                Ylp = work.tile([104, 576], F32)
                nc.gpsimd.memset(Ylp[:, 0:32], 0)
                nc.gpsimd.memset(Ylp[:, 544:576], 0)
                nc.vector.tensor_scalar_max(Ylp[:, 32:544], m[:], 0.0)

                # lat conv (along ho, free axis; contiguous slices)
                nc.vector.tensor_tensor(m[:], ul0, Ylp[:, 0:512], MULT)
                nc.vector.tensor_tensor(m2[:], ul1, Ylp[:, 32:544], MULT)
                nc.vector.tensor_tensor(m[:], m[:], m2[:], ADD)
                nc.vector.tensor_tensor(m2[:], ul2, Ylp[:, 64:576], MULT)
                nc.vector.tensor_tensor(m[:], m[:], m2[:], ADD)
                nc.vector.tensor_tensor(m[:], m[:], blat, ADD)

                # upsample into U: relu + h-expand into ws=0 (scalar),
                # then w-expand ws=1..3 from ws=0 in place (vector)
                U = outp.tile([104, SLICE], F32)  # (ho, hs, wo, ws)
                Uv = U[:].rearrange("p (ho hs wo ws) -> p ho hs wo ws",
                                    ho=16, hs=4, wo=32, ws=4)
                mb = m[:].rearrange("p (ho wo) -> p ho wo", ho=16) \
                         .unsqueeze(2).broadcast_to([104, 16, 4, 32])
                nc.scalar.activation(Uv[:, :, :, :, 0], mb, RELU)
                Uw = U[:].rearrange("p (h wo ws) -> p h wo ws", h=64, ws=4)
                src = Uw[:, :, :, 0:1].broadcast_to([104, 64, 32, 3])
                nc.vector.tensor_scalar_add(Uw[:, :, :, 1:4], src, 0.0)

                # stores on the scalar HWDGE queue; 3 interleaved depth
                # slices per group read the same partition-slice of U
                for half, g in enumerate((2 * k, 2 * k + 1)):
                    off = (g % G) * B_GRP * BSTRIDE
                    for di in range(3):
                        nc.scalar.dma_start(
                            bass.AP(y, off + di * SLICE,
                                    [[BSTRIDE, 8], [3 * SLICE, 5], [1, SLICE]]),
                            U[64 * half:64 * half + 40, :])

            # software-pipelined emission
            for r in range(reps):
                b = r * G
                load(b + 0)
                load(b + 1)
                pool(b + 0)
                load(b + 2)
                pool(b + 1)
                mm_pair(b // 2 + 0)
                load(b + 3)
                pool(b + 2)
                conv_store_pair(b // 2 + 0)
                pool(b + 3)
                mm_pair(b // 2 + 1)
                conv_store_pair(b // 2 + 1)

    nc.compile()
    return nc


_NC_CACHE = {}


def _get_nc(reps: int = 1):
    if reps not in _NC_CACHE:
        _NC_CACHE[reps] = build_nc(reps)
    return _NC_CACHE[reps]


def kernel(x, w_depth, b_depth, w_lon, b_lon, w_lat, b_lat, reps: int = 1,
           **run_kwargs):
    mm, wts = _pack_consts(w_depth, b_depth, w_lon, b_lon, w_lat, b_lat)
    xf = np.ascontiguousarray(np.asarray(x), dtype=np.float32).reshape(N_CORES, CORE_ELEMS)
    in_maps = [{"x": xf[c], "mm": mm, "wts": wts} for c in range(N_CORES)]
    nc = _get_nc(reps)
    res = run_bass_kernel_spmd(nc, in_maps, core_ids=list(range(N_CORES)), **run_kwargs)
    out = np.stack([r["y"] for r in res.results], axis=0)
    out = out.reshape(B, 15, 64, 128, 1)
    if run_kwargs:
        kernel.last_results = res
    return out


# revision 10
# speedup vs baseline: 1.1452x; 1.1452x over previous
"""Trainium2 Bass kernel for nn_LocalNetwork (avgpool3d -> 3x LocallyConnected1D -> upsample3d).

Sharding: pure data parallelism — batch 256 split as 32 per core across 8 cores.

Per-core layout (B_loc=32 batches, 4 load groups of 8, conv pairs of 2 groups):
  partition p = (bl, dslice)  [8 x 15 = 120 partitions]
  Every DMA descriptor covers a 32KB-contiguous DRAM run (one (h,w) slice).

  - avg-pool over (ws) then (hs): 3+3 tensor_tensor adds (DVE runs TT at
    ~2.4 elem/cycle vs 1 for tensor_reduce)
  - depth pool + depth-conv taps: matmuls [120 -> 40] with 0/(1/48)
    matrices (fuses the /48 mean scale and the +/-1 depth shifts)
  - two load-groups pair into [80, 512] conv tiles so the ~680ns/instr
    DVE overhead is amortized over 16 batches
  - upsample: relu + h-expand into ws=0 lanes of U (scalar engine),
    w-expand ws=1..3 in place (vector, broadcast src); depth x3
    replication is FREE — three store DMAs per group read the same
    [40, 8192] partition-slice of U into interleaved DRAM depth slices.
  - loads issue on the sync HWDGE queue, stores on the scalar (Act)
    HWDGE queue: a store waiting for compute must not head-of-line
    block later loads.
"""

import numpy as np

import concourse.bass as bass
import concourse.mybir as mybir
from concourse import bacc
from concourse.bass_utils import run_bass_kernel_spmd
from concourse.tile import TileContext

F32 = mybir.dt.float32
ADD = mybir.AluOpType.add
MULT = mybir.AluOpType.mult
RELU = mybir.ActivationFunctionType.Relu

N_CORES = 8
B = 256
B_CORE = 32          # batches per core
G = 4                # load groups per core
B_GRP = 8            # batches per group
CORE_ELEMS = B_CORE * 15 * 64 * 128  # 3,932,160
BSTRIDE = 15 * 64 * 128              # 122,880
SLICE = 64 * 128                     # 8192 elems = one (h,w) plane = 32KB


def _pack_consts(w_depth, b_depth, w_lon, b_lon, w_lat, b_lat):
    """Returns (mm [120,128] f32, wts [104,6144] f32).

    mm: three matmul lhsT tiles [120,40] (cols 0:40 dn / 40:80 mid / 80:120 up)
        out[q=(bl,dp), f] = sum_p lhsT[p=(bl,dsl), q] * P2[p, f]
        coefficient 1/48 folds the avg-pool mean.
    wts: 12 x [80,512] conv weight/bias tiles, p=(half,bl,dp), f=(ho,wo);
        identical content on partitions 0:40 and 64:104 (conv pairs;
        matmul out base partition must be 0/32/64, so half b sits at 64).
    """
    mm = np.zeros((120, 128), np.float32)
    for bl in range(8):
        for dsl in range(15):
            p = bl * 15 + dsl
            grp = dsl // 3
            for col0, dp in ((0, grp + 1), (40, grp), (80, grp - 1)):
                if 0 <= dp <= 4:
                    mm[p, col0 + bl * 5 + dp] = 1.0 / 48.0

    dp = np.arange(5)[:, None, None]
    ho = np.arange(16)[None, :, None]
    wo = np.arange(32)[None, None, :]
    ld = wo * 112 + ho * 7 + (dp + 1)     # depth seq index (5,16,32)
    ll = dp * 544 + ho * 34 + (wo + 1)    # lon
    lt = dp * 576 + wo * 18 + (ho + 1)    # lat

    def tile(vec, idx):
        t = np.broadcast_to(np.asarray(vec)[idx][None], (8, 5, 16, 32))
        return t.reshape(40, 512)

    cols = []
    for j in range(3):
        cols.append(tile(np.asarray(w_depth)[:, j], ld))
    cols.append(tile(b_depth, ld))
    for j in range(3):
        cols.append(tile(np.asarray(w_lon)[:, j], ll))
    cols.append(tile(b_lon, ll))
    for j in range(3):
        cols.append(tile(np.asarray(w_lat)[:, j], lt))
    cols.append(tile(b_lat, lt))
    wts40 = np.concatenate(cols, axis=1)
    wts = np.zeros((104, 6144), np.float32)
    wts[0:40] = wts40
    wts[64:104] = wts40
    return mm, np.ascontiguousarray(wts)


def build_nc(reps: int = 1) -> bass.Bass:
    nc = bacc.Bacc("TRN2", target_bir_lowering=False, debug=False)
    x = nc.dram_tensor("x", [CORE_ELEMS], F32, kind="ExternalInput")
    mmc = nc.dram_tensor("mm", [120, 128], F32, kind="ExternalInput")
    wtc = nc.dram_tensor("wts", [104, 6144], F32, kind="ExternalInput")
    y = nc.dram_tensor("y", [CORE_ELEMS], F32, kind="ExternalOutput")

    with TileContext(nc) as tc:
        with (
            tc.tile_pool(name="cpool", bufs=1) as cpool,
            tc.tile_pool(name="inp", bufs=2) as inp,
            tc.tile_pool(name="outp", bufs=2) as outp,
            tc.tile_pool(name="work", bufs=2) as work,
            tc.tile_pool(name="psum", bufs=2, space="PSUM") as psum,
        ):
            MM = cpool.tile([120, 128], F32)
            WT = cpool.tile([104, 6144], F32)
            nc.sync.dma_start(MM[:], mmc[:])
            nc.sync.dma_start(WT[:], wtc[:])
            w = lambda i: WT[:, i * 512:(i + 1) * 512]
            wd0, wd1, wd2, bd = (w(i) for i in range(4))
            vl0, vl1, vl2, blon = (w(i) for i in range(4, 8))
            ul0, ul1, ul2, blat = (w(i) for i in range(8, 12))

            state = {}

            def load(g):
                off = (g % G) * B_GRP * BSTRIDE
                X = inp.tile([120, SLICE], F32)
                nc.sync.dma_start(
                    X[:], bass.AP(x, off, [[BSTRIDE, 8], [SLICE, 15], [1, SLICE]]))
                state[g] = X

            def pool(g):
                X = state.pop(g)
                # h,w avg-pool (sum): one fused reduce over (hs, ws)
                P2 = work.tile([120, 512], F32)
                nc.vector.tensor_reduce(
                    P2[:].rearrange("p (ho wo) -> p ho wo", ho=16),
                    X[:].rearrange("p (ho hs wo ws) -> p ho wo hs ws",
                                   ho=16, hs=4, wo=32, ws=4),
                    mybir.AxisListType.XY, ADD)
                state[("P2", g)] = P2

            def mm_pair(k):
                # depth pool (/48) + conv taps; pair halves at partitions 0:40 / 40:80
                Sdn = psum.tile([104, 512], F32)
                S0 = psum.tile([104, 512], F32)
                Sup = psum.tile([104, 512], F32)
                for half, g in enumerate((2 * k, 2 * k + 1)):
                    P2 = state.pop(("P2", g))
                    sl = slice(64 * half, 64 * half + 40)
                    nc.tensor.matmul(Sdn[sl], MM[:, 0:40], P2[:], start=True, stop=True)
                    nc.tensor.matmul(S0[sl], MM[:, 40:80], P2[:], start=True, stop=True)
                    nc.tensor.matmul(Sup[sl], MM[:, 80:120], P2[:], start=True, stop=True)
                state[("S", k)] = (Sdn, S0, Sup)

            def conv_store_pair(k):
                Sdn, S0, Sup = state.pop(("S", k))
                # depth conv
                m = work.tile([104, 512], F32)
                m2 = work.tile([104, 512], F32)
                nc.vector.tensor_tensor(m[:], wd0, Sdn[:], MULT)
                nc.vector.tensor_tensor(m2[:], wd1, S0[:], MULT)
                nc.vector.tensor_tensor(m[:], m[:], m2[:], ADD)
                nc.vector.tensor_tensor(m2[:], wd2, Sup[:], MULT)
                nc.vector.tensor_tensor(m[:], m[:], m2[:], ADD)
                nc.vector.tensor_tensor(m[:], m[:], bd, ADD)
                # relu into lon-padded tile Ydp[p, ho*34 + (wo+1)]
                Ydp = work.tile([104, 544], F32)
                Ydpv = Ydp[:].rearrange("p (ho wp) -> p ho wp", ho=16, wp=34)
                nc.gpsimd.memset(Ydpv[:, :, 0], 0)
                nc.gpsimd.memset(Ydpv[:, :, 33], 0)
                nc.vector.tensor_scalar_max(
                    Ydpv[:, :, 1:33],
                    m[:].rearrange("p (ho wo) -> p ho wo", ho=16), 0.0)

                # lon conv (along wo, free axis)
                m3 = m[:].rearrange("p (ho wo) -> p ho wo", ho=16)
                m23 = m2[:].rearrange("p (ho wo) -> p ho wo", ho=16)
                w3 = lambda t: t.rearrange("p (ho wo) -> p ho wo", ho=16)
                nc.vector.tensor_tensor(m3, w3(vl0), Ydpv[:, :, 0:32], MULT)
                nc.vector.tensor_tensor(m23, w3(vl1), Ydpv[:, :, 1:33], MULT)
                nc.vector.tensor_tensor(m3, m3, m23, ADD)
                nc.vector.tensor_tensor(m23, w3(vl2), Ydpv[:, :, 2:34], MULT)
                nc.vector.tensor_tensor(m3, m3, m23, ADD)
                nc.vector.tensor_tensor(m3, m3, w3(blon), ADD)
                # relu into lat-padded tile Ylp[p, (ho+1)*32 + wo]
                Ylp = work.tile([104, 576], F32)
                nc.gpsimd.memset(Ylp[:, 0:32], 0)
                nc.gpsimd.memset(Ylp[:, 544:576], 0)
                nc.vector.tensor_scalar_max(Ylp[:, 32:544], m[:], 0.0)

                # lat conv (along ho, free axis; contiguous slices)
                nc.vector.tensor_tensor(m[:], ul0, Ylp[:, 0:512], MULT)
                nc.vector.tensor_tensor(m2[:], ul1, Ylp[:, 32:544], MULT)
                nc.vector.tensor_tensor(m[:], m[:], m2[:], ADD)
                nc.vector.tensor_tensor(m2[:], ul2, Ylp[:, 64:576], MULT)
                nc.vector.tensor_tensor(m[:], m[:], m2[:], ADD)
                nc.vector.tensor_tensor(m[:], m[:], blat, ADD)

                # upsample: relu + h-expand (contiguous dst), then
                # w-expand into the fully contiguous U tile (both vector,
                # unit-stride writes; ISA allows max 3 free dims per AP)
                A = work.tile([104, 2048], F32)  # (ho, hs, wo)
                Av = A[:].rearrange("p (ho hs wo) -> p ho hs wo", ho=16, hs=4)
                mb = m[:].rearrange("p (ho wo) -> p ho wo", ho=16) \
                         .unsqueeze(2).broadcast_to([104, 16, 4, 32])
                nc.vector.tensor_scalar_max(Av, mb, 0.0)
                U = outp.tile([104, SLICE], F32)  # (h, wo, ws)
                Uw = U[:].rearrange("p (h wo ws) -> p h wo ws", h=64, ws=4)
                Ab = A[:].rearrange("p (h wo) -> p h wo", h=64) \
                         .unsqueeze(3).broadcast_to([104, 64, 32, 4])
                nc.vector.tensor_scalar_add(Uw, Ab, 0.0)

                # stores on the scalar HWDGE queue; 3 interleaved depth
                # slices per group read the same partition-slice of U
                for half, g in enumerate((2 * k, 2 * k + 1)):
                    off = (g % G) * B_GRP * BSTRIDE
                    for di in range(3):
                        nc.scalar.dma_start(
                            bass.AP(y, off + di * SLICE,
                                    [[BSTRIDE, 8], [3 * SLICE, 5], [1, SLICE]]),
                            U[64 * half:64 * half + 40, :])

            # software-pipelined emission
            for r in range(reps):
                b = r * G
                load(b + 0)
                load(b + 1)
                pool(b + 0)
                load(b + 2)
                pool(b + 1)
                mm_pair(b // 2 + 0)
                load(b + 3)
                pool(b + 2)
                conv_store_pair(b // 2 + 0)
                pool(b + 3)
                mm_pair(b // 2 + 1)
                conv_store_pair(b // 2 + 1)

    nc.compile()
    return nc


_NC_CACHE = {}


def _get_nc(reps: int = 1):
    if reps not in _NC_CACHE:
        _NC_CACHE[reps] = build_nc(reps)
    return _NC_CACHE[reps]


def kernel(x, w_depth, b_depth, w_lon, b_lon, w_lat, b_lat, reps: int = 1,
           **run_kwargs):
    mm, wts = _pack_consts(w_depth, b_depth, w_lon, b_lon, w_lat, b_lat)
    xf = np.ascontiguousarray(np.asarray(x), dtype=np.float32).reshape(N_CORES, CORE_ELEMS)
    in_maps = [{"x": xf[c], "mm": mm, "wts": wts} for c in range(N_CORES)]
    nc = _get_nc(reps)
    res = run_bass_kernel_spmd(nc, in_maps, core_ids=list(range(N_CORES)), **run_kwargs)
    out = np.stack([r["y"] for r in res.results], axis=0)
    out = out.reshape(B, 15, 64, 128, 1)
    if run_kwargs:
        kernel.last_results = res
    return out


# revision 13
# speedup vs baseline: 1.3357x; 1.1663x over previous
"""Trainium2 Bass kernel for nn_LocalNetwork (avgpool3d -> 3x LocallyConnected1D -> upsample3d).

Sharding: pure data parallelism — batch 256 split as 32 per core across 8 cores.

Per-core layout (B_loc=32 batches, 4 load groups of 8, conv pairs of 2 groups):
  partition p = (bl, dslice)  [8 x 15 = 120 partitions]
  Every DMA descriptor covers a 32KB-contiguous DRAM run (one (h,w) slice).

  - avg-pool over (ws) then (hs): 3+3 tensor_tensor adds (DVE runs TT at
    ~2.4 elem/cycle vs 1 for tensor_reduce)
  - depth pool + depth-conv taps: matmuls [120 -> 40] with 0/(1/48)
    matrices (fuses the /48 mean scale and the +/-1 depth shifts)
  - two load-groups pair into [80, 512] conv tiles so the ~680ns/instr
    DVE overhead is amortized over 16 batches
  - upsample: relu + h-expand into ws=0 lanes of U (scalar engine),
    w-expand ws=1..3 in place (vector, broadcast src); depth x3
    replication is FREE — three store DMAs per group read the same
    [40, 8192] partition-slice of U into interleaved DRAM depth slices.
  - loads issue on the sync HWDGE queue, stores on the scalar (Act)
    HWDGE queue: a store waiting for compute must not head-of-line
    block later loads.
"""

import numpy as np

import concourse.bass as bass
import concourse.mybir as mybir
from concourse import bacc
from concourse.bass_utils import run_bass_kernel_spmd
from concourse.tile import TileContext

F32 = mybir.dt.float32
ADD = mybir.AluOpType.add
MULT = mybir.AluOpType.mult
RELU = mybir.ActivationFunctionType.Relu

N_CORES = 8
B = 256
B_CORE = 32          # batches per core
G = 4                # load groups per core
B_GRP = 8            # batches per group
CORE_ELEMS = B_CORE * 15 * 64 * 128  # 3,932,160
BSTRIDE = 15 * 64 * 128              # 122,880
SLICE = 64 * 128                     # 8192 elems = one (h,w) plane = 32KB


def _pack_consts(w_depth, b_depth, w_lon, b_lon, w_lat, b_lat):
    """Returns (mm [120,128] f32, wts [104,6144] f32).

    mm: three matmul lhsT tiles [120,40] (cols 0:40 dn / 40:80 mid / 80:120 up)
        out[q=(bl,dp), f] = sum_p lhsT[p=(bl,dsl), q] * P2[p, f]
        coefficient 1/48 folds the avg-pool mean.
    wts: 12 x [80,512] conv weight/bias tiles, p=(half,bl,dp), f=(ho,wo);
        identical content on partitions 0:40 and 64:104 (conv pairs;
        matmul out base partition must be 0/32/64, so half b sits at 64).
    """
    mm = np.zeros((120, 128), np.float32)
    for bl in range(8):
        for dsl in range(15):
            p = bl * 15 + dsl
            grp = dsl // 3
            for col0, dp in ((0, grp + 1), (40, grp), (80, grp - 1)):
                if 0 <= dp <= 4:
                    mm[p, col0 + bl * 5 + dp] = 1.0 / 48.0

    dp = np.arange(5)[:, None, None]
    ho = np.arange(16)[None, :, None]
    wo = np.arange(32)[None, None, :]
    ld = wo * 112 + ho * 7 + (dp + 1)     # depth seq index (5,16,32)
    ll = dp * 544 + ho * 34 + (wo + 1)    # lon
    lt = dp * 576 + wo * 18 + (ho + 1)    # lat

    def tile(vec, idx):
        t = np.broadcast_to(np.asarray(vec)[idx][None], (8, 5, 16, 32))
        return t.reshape(40, 512)

    cols = []
    for j in range(3):
        cols.append(tile(np.asarray(w_depth)[:, j], ld))
    cols.append(tile(b_depth, ld))
    for j in range(3):
        cols.append(tile(np.asarray(w_lon)[:, j], ll))
    cols.append(tile(b_lon, ll))
    for j in range(3):
        cols.append(tile(np.asarray(w_lat)[:, j], lt))
    cols.append(tile(b_lat, lt))
    wts40 = np.concatenate(cols, axis=1)
    wts = np.zeros((104, 6144), np.float32)
    wts[0:40] = wts40
    wts[64:104] = wts40
    return mm, np.ascontiguousarray(wts)


def build_nc(reps: int = 1) -> bass.Bass:
    nc = bacc.Bacc("TRN2", target_bir_lowering=False, debug=False)
    x = nc.dram_tensor("x", [CORE_ELEMS], F32, kind="ExternalInput")
    mmc = nc.dram_tensor("mm", [120, 128], F32, kind="ExternalInput")
    wtc = nc.dram_tensor("wts", [104, 6144], F32, kind="ExternalInput")
    y = nc.dram_tensor("y", [CORE_ELEMS], F32, kind="ExternalOutput")

    with TileContext(nc) as tc:
        with (
            tc.tile_pool(name="cpool", bufs=1) as cpool,
            tc.tile_pool(name="inp", bufs=2) as inp,
            tc.tile_pool(name="outp", bufs=2) as outp,
            tc.tile_pool(name="work", bufs=2) as work,
            tc.tile_pool(name="psum", bufs=2, space="PSUM") as psum,
        ):
            MM = cpool.tile([120, 128], F32)
            WT = cpool.tile([104, 6144], F32)
            nc.sync.dma_start(MM[:], mmc[:])
            nc.sync.dma_start(WT[:], wtc[:])
            w = lambda i: WT[:, i * 512:(i + 1) * 512]
            wd0, wd1, wd2, bd = (w(i) for i in range(4))
            vl0, vl1, vl2, blon = (w(i) for i in range(4, 8))
            ul0, ul1, ul2, blat = (w(i) for i in range(8, 12))

            state = {}

            def load(g):
                # two half-loads so the first half's pool reduce overlaps
                # the second half's transfer (h rows 0:32 then 32:64)
                off = (g % G) * B_GRP * BSTRIDE
                X = inp.tile([120, SLICE], F32)
                half = SLICE // 2
                for c in range(2):
                    nc.sync.dma_start(
                        X[:, c * half:(c + 1) * half],
                        bass.AP(x, off + c * half,
                                [[BSTRIDE, 8], [SLICE, 15], [1, half]]))
                state[g] = X

            def pool(g):
                X = state.pop(g)
                # h,w avg-pool (sum): fused reduce over (hs, ws), one per
                # load half (ho 0:8 | 8:16)
                P2 = work.tile([120, 512], F32)
                for c in range(2):
                    nc.vector.tensor_reduce(
                        P2[:, c * 256:(c + 1) * 256]
                            .rearrange("p (ho wo) -> p ho wo", ho=8),
                        X[:, c * 4096:(c + 1) * 4096]
                            .rearrange("p (ho hs wo ws) -> p ho wo hs ws",
                                       ho=8, hs=4, wo=32, ws=4),
                        mybir.AxisListType.XY, ADD)
                state[("P2", g)] = P2

            def mm_half(g):
                # depth pool (/48) + conv taps; pair half a at partitions
                # 0:40, half b at 64:104 (matmul out base must be 0/32/64)
                k, half = divmod(g, 2)
                if half == 0:
                    Sdn = psum.tile([104, 512], F32)
                    S0 = psum.tile([104, 512], F32)
                    Sup = psum.tile([104, 512], F32)
                    state[("S", k)] = (Sdn, S0, Sup)
                else:
                    Sdn, S0, Sup = state[("S", k)]
                P2 = state.pop(("P2", g))
                sl = slice(64 * half, 64 * half + 40)
                nc.tensor.matmul(Sdn[sl], MM[:, 0:40], P2[:], start=True, stop=True)
                nc.tensor.matmul(S0[sl], MM[:, 40:80], P2[:], start=True, stop=True)
                nc.tensor.matmul(Sup[sl], MM[:, 80:120], P2[:], start=True, stop=True)

            def conv_store_pair(k):
                Sdn, S0, Sup = state.pop(("S", k))
                # depth conv
                m = work.tile([104, 512], F32)
                m2 = work.tile([104, 512], F32)
                nc.vector.tensor_tensor(m[:], wd0, Sdn[:], MULT)
                nc.vector.tensor_tensor(m2[:], wd1, S0[:], MULT)
                nc.vector.tensor_tensor(m[:], m[:], m2[:], ADD)
                nc.vector.tensor_tensor(m2[:], wd2, Sup[:], MULT)
                nc.vector.tensor_tensor(m[:], m[:], m2[:], ADD)
                nc.vector.tensor_tensor(m[:], m[:], bd, ADD)
                # relu into lon-padded tile Ydp[p, ho*34 + (wo+1)]
                Ydp = work.tile([104, 544], F32)
                Ydpv = Ydp[:].rearrange("p (ho wp) -> p ho wp", ho=16, wp=34)
                nc.gpsimd.memset(Ydpv[:, :, 0], 0)
                nc.gpsimd.memset(Ydpv[:, :, 33], 0)
                nc.vector.tensor_scalar_max(
                    Ydpv[:, :, 1:33],
                    m[:].rearrange("p (ho wo) -> p ho wo", ho=16), 0.0)

                # lon conv (along wo, free axis)
                m3 = m[:].rearrange("p (ho wo) -> p ho wo", ho=16)
                m23 = m2[:].rearrange("p (ho wo) -> p ho wo", ho=16)
                w3 = lambda t: t.rearrange("p (ho wo) -> p ho wo", ho=16)
                nc.vector.tensor_tensor(m3, w3(vl0), Ydpv[:, :, 0:32], MULT)
                nc.vector.tensor_tensor(m23, w3(vl1), Ydpv[:, :, 1:33], MULT)
                nc.vector.tensor_tensor(m3, m3, m23, ADD)
                nc.vector.tensor_tensor(m23, w3(vl2), Ydpv[:, :, 2:34], MULT)
                nc.vector.tensor_tensor(m3, m3, m23, ADD)
                nc.vector.tensor_tensor(m3, m3, w3(blon), ADD)
                # relu into lat-padded tile Ylp[p, (ho+1)*32 + wo]
                Ylp = work.tile([104, 576], F32)
                nc.gpsimd.memset(Ylp[:, 0:32], 0)
                nc.gpsimd.memset(Ylp[:, 544:576], 0)
                nc.vector.tensor_scalar_max(Ylp[:, 32:544], m[:], 0.0)

                # lat conv (along ho, free axis; contiguous slices)
                nc.vector.tensor_tensor(m[:], ul0, Ylp[:, 0:512], MULT)
                nc.vector.tensor_tensor(m2[:], ul1, Ylp[:, 32:544], MULT)
                nc.vector.tensor_tensor(m[:], m[:], m2[:], ADD)
                nc.vector.tensor_tensor(m2[:], ul2, Ylp[:, 64:576], MULT)
                nc.vector.tensor_tensor(m[:], m[:], m2[:], ADD)
                nc.vector.tensor_tensor(m[:], m[:], blat, ADD)

                # upsample: relu + h-expand (contiguous dst), then
                # w-expand into the fully contiguous U tile (both vector,
                # unit-stride writes; ISA allows max 3 free dims per AP)
                A = work.tile([104, 2048], F32)  # (ho, hs, wo)
                Av = A[:].rearrange("p (ho hs wo) -> p ho hs wo", ho=16, hs=4)
                mb = m[:].rearrange("p (ho wo) -> p ho wo", ho=16) \
                         .unsqueeze(2).broadcast_to([104, 16, 4, 32])
                nc.vector.tensor_scalar_max(Av, mb, 0.0)
                U = outp.tile([104, SLICE], F32)  # (h, wo, ws)
                Uw = U[:].rearrange("p (h wo ws) -> p h wo ws", h=64, ws=4)
                Ab = A[:].rearrange("p (h wo) -> p h wo", h=64) \
                         .unsqueeze(3).broadcast_to([104, 64, 32, 4])
                nc.vector.tensor_scalar_add(Uw, Ab, 0.0)

                # stores split across both HWDGE queues (scalar + sync);
                # 3 interleaved depth slices per group read the same
                # partition-slice of U
                for half, g in enumerate((2 * k, 2 * k + 1)):
                    off = (g % G) * B_GRP * BSTRIDE
                    eng = nc.scalar if half == 0 else nc.sync
                    for di in range(3):
                        eng.dma_start(
                            bass.AP(y, off + di * SLICE,
                                    [[BSTRIDE, 8], [3 * SLICE, 5], [1, SLICE]]),
                            U[64 * half:64 * half + 40, :])

            # software-pipelined emission
            for r in range(reps):
                b = r * G
                load(b + 0)
                load(b + 1)
                pool(b + 0)
                mm_half(b + 0)
                load(b + 2)
                pool(b + 1)
                mm_half(b + 1)
                load(b + 3)
                pool(b + 2)
                mm_half(b + 2)
                conv_store_pair(b // 2 + 0)
                pool(b + 3)
                mm_half(b + 3)
                conv_store_pair(b // 2 + 1)

    nc.compile()
    return nc


_NC_CACHE = {}


def _get_nc(reps: int = 1):
    if reps not in _NC_CACHE:
        _NC_CACHE[reps] = build_nc(reps)
    return _NC_CACHE[reps]


def kernel(x, w_depth, b_depth, w_lon, b_lon, w_lat, b_lat, reps: int = 1,
           **run_kwargs):
    mm, wts = _pack_consts(w_depth, b_depth, w_lon, b_lon, w_lat, b_lat)
    xf = np.ascontiguousarray(np.asarray(x), dtype=np.float32).reshape(N_CORES, CORE_ELEMS)
    in_maps = [{"x": xf[c], "mm": mm, "wts": wts} for c in range(N_CORES)]
    nc = _get_nc(reps)
    res = run_bass_kernel_spmd(nc, in_maps, core_ids=list(range(N_CORES)), **run_kwargs)
    out = np.stack([r["y"] for r in res.results], axis=0)
    out = out.reshape(B, 15, 64, 128, 1)
    if run_kwargs:
        kernel.last_results = res
    return out
